# revision 1
# baseline (speedup 1.0000x reference)
"""GAT (2-layer multi-head graph attention) on 8 Trainium2 NeuronCores.

Sharding: nodes (rows of adj / attention) are sharded across the 8 cores;
each core computes h = x@W replicated, its 512-row block of
e/softmax/aggregation for both GAT layers, with an AllGather of the layer-1
output (xcat) between layers.

Layout trick: attention probabilities are computed TRANSPOSED (eT[j, i]) so
softmax-normalizer and aggregation both run on the tensor engine:
  aggT[o, i] = sum_j hplus[j, o] * P[j, i]  with hplus = [h | 1] so the last
row of the accumulator is the softmax denominator Z.  exp/leaky run on the
scalar engine (Prelu alpha=0.2 + Exp share one ACT table set), masking is a
single DVE scalar_tensor_tensor using (adj-1)*100 added before the leaky
(masked entries land at ~exp(-16) -> 0).
"""
import os
import sys

for _p in ("/opt/trn_rl_repo", "/root/.axon_site/_ro/trn_rl_repo"):
    if os.path.isdir(_p) and _p not in sys.path:
        sys.path.insert(0, _p)

import numpy as np
import ml_dtypes

import concourse.bacc as bacc
import concourse.mybir as mybir
import concourse.tile as tile
from concourse import bass_utils

F32 = mybir.dt.float32
F32R = mybir.dt.float32r
BF16 = mybir.dt.bfloat16
AF = mybir.ActivationFunctionType
ALU = mybir.AluOpType

N, NFEAT, NHID, NCLASS, NHEADS = 4096, 512, 64, 128, 8
NCORES = 8
R = N // NCORES          # 512 rows per core
FC = NFEAT // 128        # 4 feature chunks
JC = N // 128            # 32 j-chunks
BIG = 100.0
ALPHA = 0.2

_CACHE = {}


def _build_nc():
    nc = bacc.Bacc("TRN2", target_bir_lowering=False, debug=False,
                   num_devices=NCORES)

    xT_d = nc.dram_tensor("xT", [NFEAT, N], F32R, kind="ExternalInput")
    xTb_d = nc.dram_tensor("xTblk", [NFEAT, R], F32R, kind="ExternalInput")
    Wcat_d = nc.dram_tensor("Wcat", [NFEAT, 512], F32R, kind="ExternalInput")
    WcatT_d = nc.dram_tensor("WcatT", [512, NFEAT], F32R, kind="ExternalInput")
    A12_d = nc.dram_tensor("A12", [512, 16], F32R, kind="ExternalInput")
    Wout_d = nc.dram_tensor("Wout", [512, NCLASS], F32R, kind="ExternalInput")
    WoutT_d = nc.dram_tensor("WoutT", [NCLASS, 512], F32R, kind="ExternalInput")
    AO_d = nc.dram_tensor("AO", [NCLASS, 2], F32R, kind="ExternalInput")
    adj_d = nc.dram_tensor("adjm1T", [N, R], BF16, kind="ExternalInput")
    id_d = nc.dram_tensor("ident", [128, 128], F32, kind="ExternalInput")
    out_d = nc.dram_tensor("out", [R, NCLASS], F32, kind="ExternalOutput")

    with tile.TileContext(nc, num_cores=NCORES) as tc:
        with (
            tc.tile_pool(name="persist", bufs=1) as Pp,
            tc.tile_pool(name="dram", bufs=1, space="DRAM") as Pd,
            tc.tile_pool(name="psA", bufs=2, space="PSUM") as PsA,
            tc.tile_pool(name="psS", bufs=2, space="PSUM") as PsS,
            tc.tile_pool(name="pagg", bufs=1, space="PSUM") as Pagg,
        ):
            # ---- persistent constants / small state ----
            alpha = Pp.tile([128, 1], F32, name="alpha")
            nc.vector.memset(alpha[:], ALPHA)
            onescol = Pp.tile([128, 1], F32R, name="onescol")
            nc.vector.memset(onescol[:].bitcast(F32), 1.0)
            sfjT = Pp.tile([128, JC, 8], F32, name="sfjT")
            sxcb = Pp.tile([128, FC, R], F32, name="sxcb")  # own xcatT block
            sw12 = Pp.tile([128, FC, 16], F32, name="sw12")
            sWcatF = Pp.tile([128, FC, 512], F32, name="sWcatF")
            for fc in range(FC):
                nc.sync.dma_start(
                    sWcatF[:, fc, :],
                    Wcat_d.ap()[fc * 128:(fc + 1) * 128, :].bitcast(F32))
            sWout = Pp.tile([128, FC, NCLASS], F32, name="sWout")
            for fc in range(FC):
                nc.sync.dma_start(
                    sWout[:, fc, :],
                    Wout_d.ap()[fc * 128:(fc + 1) * 128, :].bitcast(F32))
            sWoutT = Pp.tile([128, 512], F32, name="sWoutT")
            nc.sync.dma_start(sWoutT[:], WoutT_d.ap().bitcast(F32))
            sAO = Pp.tile([128, 2], F32, name="sAO")
            nc.sync.dma_start(sAO[:], AO_d.ap().bitcast(F32))
            sw2 = Pp.tile([128, FC, 2], F32, name="sw2")
            for fc in range(FC):
                pw2 = PsS.tile([128, 2], F32, tag="ps_s", bufs=2)
                nc.tensor.matmul(
                    pw2[:], sWoutT[:, fc * 128:(fc + 1) * 128], sAO[:],
                    start=True, stop=True)
                nc.vector.tensor_copy(sw2[:, fc, :], pw2[:])
            fibcat = Pp.tile([128, NHEADS * R], F32, name="fibcat")

            with tc.tile_pool(name="hplusp", bufs=1) as Ph:
                shplus = Ph.tile([128, JC, NHEADS, NHID + 1], F32R, name="shplus")
                nc.vector.memset(shplus[:, :, :, NHID].bitcast(F32), 1.0)

                # ================= stage 1: weights / fifj =================
                with tc.tile_pool(name="stage1", bufs=1) as P1:
                    sfown = P1.tile([16, R], F32, name="sfown")

                    with tc.tile_pool(name="stage1a", bufs=1) as P1a:
                        sA12 = P1a.tile([128, 4, 16], F32, name="sA12")
                        for hoc in range(4):
                            nc.sync.dma_start(
                                sA12[:, hoc, :],
                                A12_d.ap()[hoc * 128:(hoc + 1) * 128, :].bitcast(F32))
                        sxTb = P1a.tile([128, FC, R], F32, name="sxTb")
                        for fc in range(FC):
                            nc.sync.dma_start(
                                sxTb[:, fc, :],
                                xTb_d.ap()[fc * 128:(fc + 1) * 128, :].bitcast(F32))

                        # w12[f, k] = sum_ho WcatT[ho, f] * A12[ho, k]
                        # 4 parallel slab DMAs up front, then back-to-back mms
                        sWcT = P1a.tile([128, 4, NFEAT], F32, name="sWcT")
                        for hoc in range(4):
                            nc.sync.dma_start(
                                sWcT[:, hoc, :],
                                WcatT_d.ap()[hoc * 128:(hoc + 1) * 128, :]
                                .bitcast(F32))
                        for fc in range(FC):
                            pw = PsS.tile([128, 16], F32, tag="ps_s", bufs=2)
                            for hoc in range(4):
                                nc.tensor.matmul(
                                    pw[:],
                                    sWcT[:, hoc, fc * 128:(fc + 1) * 128],
                                    sA12[:, hoc, :],
                                    start=(hoc == 0), stop=(hoc == 3))
                            nc.vector.tensor_copy(sw12[:, fc, :], pw[:])

                        def prep_jc(jc):
                            """stage-A hplus[jc] + fj columns[jc], exact fp32,
                            streaming x tiles from DRAM."""
                            xa = []
                            for fc in range(FC):
                                t = Pp.tile([128, 128], F32, tag=f"xa{fc}",
                                            bufs=2, name=f"xa{fc}_{jc}")
                                nc.sync.dma_start(
                                    t[:], xT_d.ap()[fc * 128:(fc + 1) * 128,
                                                    jc * 128:(jc + 1) * 128]
                                    .bitcast(F32))
                                xa.append(t)
                            pA = PsA.tile([128, 512], F32, tag="ps_a", bufs=2,
                                          name=f"pA{jc}")
                            for fc in range(FC):
                                nc.tensor.matmul(
                                    pA[:], xa[fc][:], sWcatF[:, fc, :],
                                    start=(fc == 0), stop=(fc == 3))
                            nc.vector.tensor_copy(
                                shplus[:, jc, :, 0:NHID],
                                pA[:].rearrange("p (hd o) -> p hd o", o=NHID))
                            pfj = PsS.tile([128, 8], F32, tag="ps_s", bufs=2,
                                           name=f"pfj{jc}")
                            for fc in range(FC):
                                nc.tensor.matmul(
                                    pfj[:], xa[fc][:], sw12[:, fc, 0:8],
                                    start=(fc == 0), stop=(fc == 3))
                            nc.vector.tensor_copy(sfjT[:, jc, :], pfj[:])


                        prep_jc(0)
                        prep_jc(1)

                        # own-block fifj (for fi of this core's rows)
                        pfo = PsS.tile([16, 512], F32, tag="ps_s", bufs=2)
                        for fc in range(FC):
                            nc.tensor.matmul(
                                pfo[:], sw12[:, fc, :], sxTb[:, fc, :],
                                start=(fc == 0), stop=(fc == 3))
                        nc.vector.tensor_copy(sfown[:], pfo[:])

                    # all 8 fi rows -> one [1, 8*R] row, one broadcast;
                    # fib[hd] is then a free-dim slice of fibcat
                    fcat = P1.tile([1, NHEADS * R], F32, name="fcat")
                    nc.gpsimd.dma_start(
                        fcat[:].rearrange("o (hd r) -> o hd r", hd=NHEADS),
                        sfown[8:16, :].rearrange("hd r -> () hd r")
                        if False else sfown[8:16, :])
                    nc.gpsimd.partition_broadcast(fibcat[:], fcat[:])

                # ================= layer-1 attention sweeps =================
                with tc.tile_pool(name="chunkL1", bufs=1) as Pc:
                    paggs = {}
                    for sweep in range(2):
                        heads = list(range(sweep * 4, sweep * 4 + 4))
                        for jc in range(JC):
                            if sweep == 0 and jc + 2 < JC:
                                prep_jc(jc + 2)
                            mask = Pc.tile([128, 512], BF16, tag="mask", bufs=3)
                            nc.sync.dma_start(
                                mask[:], adj_d.ap()[jc * 128:(jc + 1) * 128, :])
                            raw4 = Pc.tile([128, 2048], F32, tag="raw4", bufs=2)
                            em4 = Pc.tile([128, 2048], F32, tag="em4", bufs=3)
                            P4 = Pc.tile([128, 2048], F32R, tag="p4", bufs=2)
                            for q, hd in enumerate(heads):
                                sl = slice(q * 512, (q + 1) * 512)
                                gidx = (sweep * JC + jc) * 4 + q
                                nc.vector.scalar_tensor_tensor(
                                    raw4[:, sl], mask[:], BIG,
                                    fibcat[:, hd * R:(hd + 1) * R],
                                    op0=ALU.mult, op1=ALU.add)
                                if (gidx * 7) % 26 < 7:
                                    u = Pc.tile([128, 512], F32, tag="ulk",
                                                bufs=3)
                                    nc.vector.tensor_scalar_add(
                                        u[:], raw4[:, sl],
                                        sfjT[:, jc, hd:hd + 1])
                                    nc.vector.scalar_tensor_tensor(
                                        em4[:, sl], u[:], ALPHA, u[:],
                                        op0=ALU.mult, op1=ALU.max)
                                else:
                                    nc.scalar.activation(
                                        em4[:, sl], raw4[:, sl], AF.Prelu,
                                        bias=sfjT[:, jc, hd:hd + 1],
                                        alpha=alpha[:])
                            nc.scalar.activation(P4[:], em4[:], AF.Exp)
                            for q, hd in enumerate(heads):
                                if jc == 0:
                                    paggs[hd] = Pagg.tile(
                                        [NHID + 1, 512], F32, tag=f"agg{q}",
                                        bufs=1, name=f"agg_s{sweep}_{q}")
                                nc.tensor.matmul(
                                    paggs[hd][:], shplus[:, jc, hd, :],
                                    P4[:, q * 512:(q + 1) * 512],
                                    start=(jc == 0), stop=(jc == JC - 1))

                        # normalize this sweep's heads into the xcatT block
                        zsw = Pc.tile([4, R], F32, tag="zsw", bufs=2)
                        for q, hd in enumerate(heads):
                            zst = Pc.tile([NHID + 1, R], F32, tag="zst", bufs=2)
                            nc.vector.tensor_copy(
                                zst[NHID:NHID + 1, :], paggs[hd][NHID:NHID + 1, :])
                            nc.gpsimd.dma_start(
                                zsw[q:q + 1, :], zst[NHID:NHID + 1, :])
                        rzw = Pc.tile([4, R], F32, tag="rzw", bufs=2)
                        rzs = Pc.tile([4, R], F32, tag="rzs", bufs=2)
                        nc.vector.reciprocal_approx_accurate(
                            rzw[:], zsw[:], rzs[:])
                        for q, hd in enumerate(heads):
                            rzt = Pc.tile([1, R], F32, tag="rzt", bufs=2)
                            nc.gpsimd.dma_start(rzt[:], rzw[q:q + 1, :])
                            zb = Pc.tile([64, R], F32, tag="zb", bufs=2)
                            nc.gpsimd.partition_broadcast(zb[:], rzt[:])
                            xcn = Pc.tile([64, R], F32, tag="xcn", bufs=2)
                            nc.vector.tensor_mul(
                                xcn[:], paggs[hd][0:NHID, :], zb[:])
                            nc.gpsimd.dma_start(
                                sxcb[64 * (hd % 2):64 * (hd % 2) + 64,
                                     hd // 2, :], xcn[:])

            # ===== layer-2 projections on the OWN block, then small gather =====
            # h2_block[n, c] = sum_f xcat_blk[n, f] Wout[f, c]   (own 512 nodes)
            # fifj2_block = w2.T @ xcat_blkT  -> fi2 (row 0, local), fj2 (row 1)
            dblk2 = Pd.tile([R, NCLASS], F32, name="dblk2")
            dgath2 = Pd.tile([N, NCLASS], F32, name="dgath2",
                             addr_space="Shared")
            dblk2b = Pd.tile([1, R], F32, name="dblk2b")
            dgath2b = Pd.tile([8, R], F32, name="dgath2b",
                              addr_space="Shared")
            sfo2 = Pp.tile([2, R], F32, name="sfo2")
            pf2o = PsS.tile([2, 512], F32, tag="ps_s", bufs=2)
            for fc in range(FC):
                nc.tensor.matmul(
                    pf2o[:], sw2[:, fc, :], sxcb[:, fc, :],
                    start=(fc == 0), stop=(fc == 3))
            nc.vector.tensor_copy(sfo2[:], pf2o[:])
            nc.gpsimd.dma_start(dblk2b[:], sfo2[1:2, :])
            for nc4 in range(4):
                pH = PsA.tile([128, 512], F32, tag="ps_a", bufs=2)
                for fc in range(FC):
                    nc.tensor.matmul(
                        pH[:, 0:NCLASS],
                        sxcb[:, fc, nc4 * 128:(nc4 + 1) * 128],
                        sWout[:, fc, :],
                        start=(fc == 0), stop=(fc == 3))
                sh2b = Pp.tile([128, NCLASS], F32, tag="sh2b", bufs=2)
                nc.vector.tensor_copy(sh2b[:], pH[:, 0:NCLASS])
                nc.gpsimd.dma_start(
                    dblk2[nc4 * 128:(nc4 + 1) * 128, :], sh2b[:])
            nc.gpsimd.collective_compute(
                "AllGather", ALU.bypass,
                replica_groups=[list(range(NCORES))],
                ins=[dblk2b[:].opt()], outs=[dgath2b[:].opt()])
            nc.gpsimd.collective_compute(
                "AllGather", ALU.bypass,
                replica_groups=[list(range(NCORES))],
                ins=[dblk2[:].opt()], outs=[dgath2[:].opt()])

            # ======================== layer 2 ========================
            with tc.tile_pool(name="stage2", bufs=1) as P2:
                sfj2T = P2.tile([128, JC], F32, name="sfj2T")
                nc.gpsimd.dma_start(
                    sfj2T[:].rearrange("p (r jc) -> p r jc", r=8),
                    dgath2b[:].rearrange("r (jc p) -> p r jc", p=128))
                fib2 = P2.tile([128, R], F32, name="fib2")
                nc.gpsimd.partition_broadcast(fib2[:], sfo2[0:1, :])
                sh2r = P2.tile([128, JC, NCLASS], F32R, name="sh2r")
                for jc in range(JC):
                    nc.gpsimd.dma_start(
                        sh2r[:, jc, :],
                        dgath2[jc * 128:(jc + 1) * 128, :].bitcast(F32R))
                ident = P2.tile([128, 128], F32, name="ident")
                nc.sync.dma_start(ident[:], id_d.ap())


                # layer-2 attention chunks (batch 4 jc per Exp)
                pagg2 = Pagg.tile([128, 512], F32, tag="agg0", bufs=1)
                pZ2 = Pagg.tile([1, 512], F32, tag="agg1", bufs=1)
                for jb in range(8):
                    raw4 = P2.tile([128, 2048], F32, tag="raw4b", bufs=3)
                    em4 = P2.tile([128, 2048], F32, tag="em4b", bufs=3)
                    P4 = P2.tile([128, 2048], F32R, tag="p4b", bufs=7)
                    for q in range(4):
                        jc = jb * 4 + q
                        sl = slice(q * 512, (q + 1) * 512)
                        mask = P2.tile([128, 512], BF16, tag="maskb", bufs=3)
                        nc.sync.dma_start(
                            mask[:], adj_d.ap()[jc * 128:(jc + 1) * 128, :])
                        nc.vector.scalar_tensor_tensor(
                            raw4[:, sl], mask[:], BIG, fib2[:],
                            op0=ALU.mult, op1=ALU.add)
                        if (jc * 7) % 26 < 7:
                            u = P2.tile([128, 512], F32, tag="ulk2", bufs=3)
                            nc.vector.tensor_scalar_add(
                                u[:], raw4[:, sl], sfj2T[:, jc:jc + 1])
                            nc.vector.scalar_tensor_tensor(
                                em4[:, sl], u[:], ALPHA, u[:],
                                op0=ALU.mult, op1=ALU.max)
                        else:
                            nc.scalar.activation(
                                em4[:, sl], raw4[:, sl], AF.Prelu,
                                bias=sfj2T[:, jc:jc + 1], alpha=alpha[:])
                    nc.scalar.activation(P4[:], em4[:], AF.Exp)
                    for q in range(4):
                        jc = jb * 4 + q
                        sl = slice(q * 512, (q + 1) * 512)
                        nc.tensor.matmul(
                            pagg2[:], sh2r[:, jc, :], P4[:, sl],
                            start=(jc == 0), stop=(jc == JC - 1))
                        nc.tensor.matmul(
                            pZ2[:], onescol[:], P4[:, sl],
                            start=(jc == 0), stop=(jc == JC - 1))

                # normalize, elu (per 64-class half), then transpose
                sz2 = P2.tile([1, R], F32, name="sz2")
                nc.vector.tensor_copy(sz2[:], pZ2[0:1, :])
                srz2 = P2.tile([1, R], F32, name="srz2")
                srz2s = P2.tile([1, R], F32, name="srz2s")
                nc.vector.reciprocal_approx_accurate(
                    srz2[:], sz2[:], srz2s[:])
                zb2 = P2.tile([64, R], F32, name="zb2")
                nc.gpsimd.partition_broadcast(zb2[:], srz2[:], channels=64)
                halves = []
                for nmh, pg in (("a", pagg2[0:64, :]), ("c", pagg2[64:128, :])):
                    sv = P2.tile([64, R], F32, tag="sv", bufs=1,
                                 name=f"sv{nmh}")
                    nc.vector.tensor_mul(sv[:], pg, zb2[:])
                    smin = P2.tile([64, R], F32, tag="smin", bufs=1,
                                   name=f"smin{nmh}")
                    nc.vector.tensor_scalar_min(smin[:], sv[:], 0.0)
                    sex = P2.tile([64, R], F32, tag="sex", bufs=1,
                                  name=f"sex{nmh}")
                    nc.scalar.activation(sex[:], smin[:], AF.Exp)
                    srel = P2.tile([64, R], F32, tag="srel", bufs=1,
                                   name=f"srel{nmh}")
                    nc.scalar.activation(srel[:], sv[:], AF.Relu)
                    sres = P2.tile([64, R], F32, tag=f"sres{nmh}", bufs=1,
                                   name=f"sres{nmh}")
                    nc.vector.scalar_tensor_tensor(
                        sres[:], sex[:], -1.0, srel[:],
                        op0=ALU.add, op1=ALU.add)
                    halves.append(sres)

                sts, negmxs, ssums = [], [], []
                for it in range(4):
                    st = P2.tile([128, 128], F32, tag="st", bufs=4,
                                 name=f"st{it}")
                    for q, sres in enumerate(halves):
                        ptp = PsS.tile([128, 64], F32, tag="ps_s", bufs=2,
                                       name=f"ptp{it}_{q}")
                        nc.tensor.transpose(
                            ptp[:], sres[:, it * 128:(it + 1) * 128],
                            ident[0:64, 0:64])
                        nc.vector.tensor_copy(
                            st[:, q * 64:(q + 1) * 64], ptp[:])
                    mx = P2.tile([128, 1], F32, tag="mx", bufs=4,
                                 name=f"mx{it}")
                    nc.vector.tensor_reduce(
                        mx[:], st[:], axis=mybir.AxisListType.X, op=ALU.max)
                    negmx = P2.tile([128, 1], F32, tag="negmx", bufs=4,
                                    name=f"negmx{it}")
                    nc.vector.tensor_scalar_mul(negmx[:], mx[:], -1.0)
                    sts.append(st); negmxs.append(negmx)
                for it in range(4):
                    sexp = P2.tile([128, 128], F32, tag="sexp", bufs=2,
                                   name=f"sexp{it}")
                    ssum = P2.tile([128, 1], F32, tag="ssum", bufs=4,
                                   name=f"ssum{it}")
                    nc.scalar.activation(
                        sexp[:], sts[it][:], AF.Exp, bias=negmxs[it][:],
                        accum_out=ssum[:])
                    ssums.append(ssum)
                slns = []
                for it in range(4):
                    sln = P2.tile([128, 1], F32, tag="sln", bufs=4,
                                  name=f"sln{it}")
                    nc.scalar.activation(sln[:], ssums[it][:], AF.Ln)
                    slns.append(sln)
                for it in range(4):
                    b2 = P2.tile([128, 1], F32, tag="b2", bufs=4,
                                 name=f"b2{it}")
                    nc.vector.tensor_sub(b2[:], negmxs[it][:], slns[it][:])
                    sout = P2.tile([128, 128], F32, tag="sout", bufs=2,
                                   name=f"sout{it}")
                    nc.scalar.activation(sout[:], sts[it][:], AF.Identity,
                                         bias=b2[:])
                    nc.sync.dma_start(
                        out_d.ap()[it * 128:(it + 1) * 128, :], sout[:])

    nc.finalize()
    return nc


def _get_nc():
    if "nc" not in _CACHE:
        _CACHE["nc"] = _build_nc()
    return _CACHE["nc"]


def kernel(**inputs):
    x = np.asarray(inputs["x"], dtype=np.float32)
    adj = np.asarray(inputs["adj"])
    W = np.asarray(inputs["W"], dtype=np.float32)
    a = np.asarray(inputs["a"], dtype=np.float32)
    W_out = np.asarray(inputs["W_out"], dtype=np.float32)
    a_out = np.asarray(inputs["a_out"], dtype=np.float32)

    xT = np.ascontiguousarray(x.T)
    Wcat = np.ascontiguousarray(W.transpose(1, 0, 2).reshape(NFEAT, 512))
    WcatT = np.ascontiguousarray(Wcat.T)
    A12 = np.zeros((512, 16), np.float32)
    for hd in range(NHEADS):
        A12[hd * NHID:(hd + 1) * NHID, hd] = a[hd, NHID:]      # a2 -> fj
        A12[hd * NHID:(hd + 1) * NHID, 8 + hd] = a[hd, :NHID]  # a1 -> fi
    WoutT = np.ascontiguousarray(W_out.T)
    AO = np.stack([a_out[:NCLASS], a_out[NCLASS:]], axis=1)
    AO = np.ascontiguousarray(AO, dtype=np.float32)
    ident = np.eye(128, dtype=np.float32)
    adjm1 = adj.astype(np.float32) - 1.0

    in_maps = []
    for c in range(NCORES):
        r0, r1 = c * R, (c + 1) * R
        in_maps.append({
            "xT": xT,
            "xTblk": np.ascontiguousarray(x[r0:r1].T),
            "Wcat": Wcat,
            "WcatT": WcatT,
            "A12": A12,
            "Wout": W_out,
            "WoutT": WoutT,
            "AO": AO,
            "adjm1T": np.ascontiguousarray(adjm1[r0:r1].T).astype(
                ml_dtypes.bfloat16),
            "ident": ident,
        })

    nc = _get_nc()
    trace = bool(os.environ.get("KERNEL_TRACE"))
    res = bass_utils.run_bass_kernel_spmd(
        nc, in_maps, list(range(NCORES)), trace=trace)
    kernel.last_results = res
    out = np.concatenate(
        [res.results[c]["out"] for c in range(NCORES)], axis=0)
    return np.ascontiguousarray(out, dtype=np.float32)



# revision 34
# speedup vs baseline: 1.5144x; 1.5144x over previous
"""GAT (2-layer multi-head graph attention) on 8 Trainium2 NeuronCores.

Sharding: nodes (rows of adj / attention) sharded across 8 cores; h = x@W
computed replicated in bf16; each core does its 512-row block of the
e/softmax/aggregation for both layers; one bf16 AllGather of [h2 | fj2]
between layers.

e-chain (per 128-j x 512-i slice, transposed eT[j,i] so softmax-Z and
aggregation ride the tensor engine via an hplus=[h|1] ones-row):
  u  = mask*100 + fi          (tensor_tensor add, bf16, mask broadcast x4)
  em = leaky(u + fj)          (ACT Prelu bias=fj, or DVE ts_add/ts_mul/tt_max)
  P  = exp(em) ~= bitcast_bf16(int16(184.665*em + 16250.5))   (one DVE
       tensor_scalar with int16 output -- Schraudolph exp, rel err ~4e-3
       after softmax)
Work is split across DVE/ACT/Pool engines per-group to balance occupancy.
"""
import os
import sys

for _p in ("/opt/trn_rl_repo", "/root/.axon_site/_ro/trn_rl_repo"):
    if os.path.isdir(_p) and _p not in sys.path:
        sys.path.insert(0, _p)

import numpy as np
import ml_dtypes

import concourse.bacc as bacc
import concourse.bass as bass
import concourse.mybir as mybir
import concourse.tile as tile
from concourse import bass_utils

F32 = mybir.dt.float32
BF16 = mybir.dt.bfloat16
I16 = mybir.dt.int16
AF = mybir.ActivationFunctionType
ALU = mybir.AluOpType

N, NFEAT, NHID, NCLASS, NHEADS = 4096, 512, 64, 128, 8
NCORES = 8
R = N // NCORES          # 512 rows per core
FC = NFEAT // 128        # 4 feature chunks
JC = N // 128            # 32 j-chunks
BIG = 100.0
ALPHA = 0.2
EXP_A = 184.66500888     # 128 / ln(2)
EXP_B = 16250.5          # 127*128 - schraudolph correction

# per-jc group types: 'a' = DVE u4 + ACT leaky, 'b' = Pool u4 + ACT leaky,
# 'd' = Pool u4 + DVE leaky  (balances DVE/ACT/Pool occupancy)
PTN = ['a', 'a', 'a', 'd', 'a', 'a', 'a', 'a']
PTN2 = ['a', 'a', 'a', 'd', 'a', 'a', 'a', 'd']

_CACHE = {}


def _bcast4(mask_ap, like_ap):
    """mask [128, 512] broadcast against like [128, 4, 512]."""
    _, m = bass.broadcast_tensor_aps(like_ap, mask_ap.rearrange("p i -> p () i"))
    return m


def _build_nc():
    nc = bacc.Bacc("TRN2", target_bir_lowering=False, debug=False,
                   num_devices=NCORES)

    xT4_d = nc.dram_tensor("xT4", [128, FC, N], BF16, kind="ExternalInput")
    xTb_d = nc.dram_tensor("xTblk", [NFEAT, R], BF16, kind="ExternalInput")
    Wcat_d = nc.dram_tensor("Wcat", [NFEAT, 512], BF16, kind="ExternalInput")
    WcatT_d = nc.dram_tensor("WcatT", [512, NFEAT], BF16, kind="ExternalInput")
    A12_d = nc.dram_tensor("A12", [512, 16], BF16, kind="ExternalInput")
    Wout_d = nc.dram_tensor("Wout", [512, NCLASS], BF16, kind="ExternalInput")
    WoutT_d = nc.dram_tensor("WoutT", [NCLASS, 512], BF16, kind="ExternalInput")
    AO_d = nc.dram_tensor("AO", [NCLASS, 2], BF16, kind="ExternalInput")
    adj_d = nc.dram_tensor("adjm1T", [N, R], BF16, kind="ExternalInput")
    id_d = nc.dram_tensor("ident", [128, 128], F32, kind="ExternalInput")
    out_d = nc.dram_tensor("out", [R, NCLASS], F32, kind="ExternalOutput")

    with tile.TileContext(nc, num_cores=NCORES) as tc:
        with (
            tc.tile_pool(name="persist", bufs=1) as Pp,
            tc.tile_pool(name="dram", bufs=1, space="DRAM") as Pd,
            tc.tile_pool(name="psA", bufs=2, space="PSUM") as PsA,
            tc.tile_pool(name="psS", bufs=2, space="PSUM") as PsS,
            tc.tile_pool(name="pagg", bufs=1, space="PSUM") as Pagg,
        ):
            # ---------------- persistent constants / state ----------------
            # split gather: h2 partial A (heads 0-3) gathered during sweep 1
            dblk2a = Pd.tile([R, NCLASS], BF16, name="dblk2a")
            dgath2a = Pd.tile([N, NCLASS], BF16, name="dgath2a",
                              addr_space="Shared")
            dblk2b = Pd.tile([R, NCLASS], BF16, name="dblk2b")
            dgath2b = Pd.tile([N, NCLASS], BF16, name="dgath2b",
                              addr_space="Shared")
            dblkF = Pd.tile([1, R], BF16, name="dblkF")
            dgathF = Pd.tile([NCORES, R], BF16, name="dgathF",
                             addr_space="Shared")
            alpha = Pp.tile([128, 1], F32, name="alpha")
            nc.vector.memset(alpha[:], ALPHA)
            smask = Pp.tile([128, JC, R], BF16, name="smask")
            for mq in range(8):
                nc.scalar.dma_start(
                    smask[:, mq * 4:(mq + 1) * 4, :],
                    adj_d.ap().rearrange("(jc p) i -> p jc i", p=128)
                    [:, mq * 4:(mq + 1) * 4, :])
            sWout = Pp.tile([128, FC, NCLASS], BF16, name="sWout")
            nc.sync.dma_start(
                sWout[:], Wout_d.ap().rearrange("(fc p) c -> p fc c", p=128))
            sxcb = Pp.tile([128, FC, R], BF16, name="sxcb")
            sw2 = Pp.tile([128, FC, 2], BF16, name="sw2")

            # l1-scoped pool opens here so its tiles free up SBUF for l2
            l1pool = tc.tile_pool(name="l1", bufs=1)
            P1 = l1pool.__enter__()
            shplus = P1.tile([128, JC, NHEADS, NHID + 1], BF16, name="shplus")
            nc.vector.memset(shplus[:, :, :, NHID], 1.0)
            sWcatF = P1.tile([128, FC, 512], BF16, name="sWcatF")
            nc.sync.dma_start(
                sWcatF[:], Wcat_d.ap().rearrange("(fc p) o -> p fc o", p=128))
            sfjT = P1.tile([128, JC, 8], F32, name="sfjT")
            fibcat = P1.tile([128, NHEADS * R], BF16, name="fibcat")
            sw12 = P1.tile([128, FC, 16], BF16, name="sw12")

            # ---------------- prologue: w12, w2, fi/fj --------------------
            with tc.tile_pool(name="prolog", bufs=1) as P0:
                sWoutT = P0.tile([128, 512], BF16, name="sWoutT")
                nc.sync.dma_start(sWoutT[:], WoutT_d.ap())
                sAO = P0.tile([128, 2], BF16, name="sAO")
                nc.sync.dma_start(sAO[:], AO_d.ap())
                sA12 = P0.tile([128, 4, 16], BF16, name="sA12")
                nc.sync.dma_start(
                    sA12[:], A12_d.ap().rearrange("(q p) k -> p q k", p=128))
                sxTb = P0.tile([128, FC, R], BF16, name="sxTb")
                nc.sync.dma_start(
                    sxTb[:], xTb_d.ap().rearrange("(fc p) r -> p fc r", p=128))
                sWcT = P0.tile([128, 4, NFEAT], BF16, name="sWcT")
                nc.sync.dma_start(
                    sWcT[:], WcatT_d.ap().rearrange("(q p) f -> p q f", p=128))

                # w12[f, k] = sum_ho WcatT[ho, f] * A12[ho, k]
                for fc in range(FC):
                    pw = PsS.tile([128, 16], F32, tag="ps_s", bufs=2)
                    for q in range(4):
                        nc.tensor.matmul(
                            pw[:], sWcT[:, q, fc * 128:(fc + 1) * 128],
                            sA12[:, q, :], start=(q == 0), stop=(q == 3))
                    nc.vector.tensor_copy(sw12[:, fc, :], pw[:])
                # w2[f, k] = sum_c WoutT[c, f] * AO[c, k]
                for fc in range(FC):
                    pw2 = PsS.tile([128, 2], F32, tag="ps_s", bufs=2)
                    nc.tensor.matmul(
                        pw2[:], sWoutT[:, fc * 128:(fc + 1) * 128], sAO[:],
                        start=True, stop=True)
                    nc.vector.tensor_copy(sw2[:, fc, :], pw2[:])

                # own-block fi/fj: sfown[k, r], rows 8..15 = fi per head
                pfo = PsS.tile([16, R], F32, tag="ps_s", bufs=2)
                for fc in range(FC):
                    nc.tensor.matmul(
                        pfo[:], sw12[:, fc, :], sxTb[:, fc, :],
                        start=(fc == 0), stop=(fc == 3))
                sfownb = P0.tile([16, R], BF16, name="sfownb")
                nc.vector.tensor_copy(sfownb[:], pfo[:])
                fcat = P0.tile([1, NHEADS * R], BF16, name="fcat")
                nc.gpsimd.dma_start(
                    fcat[:].rearrange("o (hd r) -> o hd r", hd=NHEADS),
                    sfownb[8:16, :])
                nc.gpsimd.partition_broadcast(fibcat[:], fcat[:])

            # h + fj streaming prep (pipelined into sweep 0)
            def prep_jc(jc, cpeng):
                xa = P1.tile([128, FC, 128], BF16, tag="xa", bufs=3,
                             name=f"xa{jc}")
                nc.sync.dma_start(xa[:], xT4_d.ap()[:, :, jc * 128:(jc + 1) * 128])
                pA = PsA.tile([128, 512], F32, tag="ps_a", bufs=2,
                              name=f"pA{jc}")
                for fc in range(FC):
                    nc.tensor.matmul(
                        pA[:], xa[:, fc, :], sWcatF[:, fc, :],
                        start=(fc == 0), stop=(fc == 3))
                pfj = PsS.tile([128, 8], F32, tag="ps_s", bufs=2,
                               name=f"pfj{jc}")
                for fc in range(FC):
                    nc.tensor.matmul(
                        pfj[:], xa[:, fc, :], sw12[:, fc, 0:8],
                        start=(fc == 0), stop=(fc == 3))
                dst = shplus[:, jc, :, 0:NHID]
                src = pA[:].rearrange("p (hd o) -> p hd o", o=NHID)
                if jc % 2 == 0:
                    nc.vector.tensor_copy(dst, src)
                else:
                    nc.scalar.activation(dst, src, AF.Copy)
                nc.vector.tensor_copy(sfjT[:, jc, :], pfj[:])

            # ---------------- layer-1 attention sweeps --------------------
            if True:
                prep_jc(0, 0)
                prep_jc(1, 1)
                paggs = {}

                def echain(u4, em4, P4, fib_ap, mask_ap, biases, gt,
                           pre=None):
                    """u4/em4: [128,4,512] bf16 (em4 may be u4), P4 bf16;
                    biases: list of 4 [128,1] f32 aps. Emits the e-chain for
                    one 4-slice group of type gt; `pre` emits mid-group."""
                    mb = _bcast4(mask_ap, fib_ap)
                    if gt in ('a',):
                        nc.vector.tensor_add(u4[:], fib_ap, mb)
                    else:
                        nc.gpsimd.tensor_add(u4[:], fib_ap, mb)
                    if pre is not None:
                        pre()
                    if gt in ('a', 'b'):
                        for q in range(4):
                            nc.scalar.activation(
                                em4[:, q, :], u4[:, q, :], AF.Prelu,
                                bias=biases[q], alpha=alpha[:])
                    else:
                        w14 = P1.tile([128, 4, 512], BF16, tag="w14", bufs=2)
                        for q in range(4):
                            nc.vector.tensor_scalar_add(
                                w14[:, q, :], u4[:, q, :], biases[q])
                        w24 = P1.tile([128, 4, 512], BF16, tag="w24", bufs=2)
                        nc.vector.tensor_scalar_mul(w24[:], w14[:], ALPHA)
                        nc.vector.tensor_max(em4[:], w14[:], w24[:])
                    nc.vector.tensor_scalar(
                        P4[:].bitcast(I16), em4[:], EXP_A, EXP_B,
                        op0=ALU.mult, op1=ALU.add)

                for sweep in range(2):
                    heads = list(range(sweep * 4, sweep * 4 + 4))
                    for jc in range(JC):
                        gt = PTN[jc % len(PTN)]
                        u4 = P1.tile([128, 4, 512], BF16, tag="u4", bufs=4)
                        em4 = P1.tile([128, 4, 512], BF16, tag="em4", bufs=3)
                        P4 = P1.tile([128, 4, 512], BF16, tag="p4", bufs=4)
                        pre = None
                        if sweep == 0 and jc + 2 < JC:
                            pre = (lambda j=jc + 2: prep_jc(j, (j % 3)))
                        echain(u4, em4, P4,
                               fibcat[:, sweep * 2048:(sweep + 1) * 2048]
                               .rearrange("p (q i) -> p q i", q=4),
                               smask[:, jc, :],
                               [sfjT[:, jc, hd:hd + 1] for hd in heads],
                               gt, pre=pre)
                        for q, hd in enumerate(heads):
                            if jc == 0:
                                paggs[hd] = Pagg.tile(
                                    [NHID + 1, 512], F32, tag=f"agg{q}",
                                    bufs=1, name=f"agg_s{sweep}_{q}")
                            nc.tensor.matmul(
                                paggs[hd][:], shplus[:, jc, hd, :],
                                P4[:, q, :],
                                start=(jc == 0), stop=(jc == JC - 1))

                    # normalize this sweep's heads into sxcb (xcatT, bf16)
                    for q, hd in enumerate(heads):
                        zq = P1.tile([1, R], F32, tag="zq", bufs=4)
                        nc.vector.tensor_copy(
                            zq[:], paggs[hd][NHID:NHID + 1, :])
                        rzq = P1.tile([1, R], F32, tag="rzq", bufs=4)
                        rzs = P1.tile([1, R], F32, tag="rzs", bufs=2)
                        nc.vector.reciprocal_approx_accurate(
                            rzq[:], zq[:], rzs[:])
                        zb = P1.tile([64, R], F32, tag="zb", bufs=2)
                        nc.gpsimd.partition_broadcast(zb[:], rzq[:])
                        nc.vector.tensor_mul(
                            sxcb[64 * (hd % 2):64 * (hd % 2) + 64,
                                 hd // 2, :],
                            paggs[hd][0:NHID, :], zb[:])

                    # fj2/fi2 chain first after sweep 1 so the tiny fj2
                    # gather fires before the big h2b gather
                    if sweep == 1:
                        pf2 = PsS.tile([2, R], F32, tag="ps_s", bufs=2)
                        for fc in range(FC):
                            nc.tensor.matmul(
                                pf2[:], sw2[:, fc, :], sxcb[:, fc, :],
                                start=(fc == 0), stop=(fc == 3))
                        sf2 = Pp.tile([2, R], BF16, name="sf2")
                        nc.vector.tensor_copy(sf2[:], pf2[:])
                        sfi2 = Pp.tile([1, R], F32, name="sfi2")
                        nc.vector.tensor_copy(sfi2[:], pf2[0:1, :])
                        nc.sync.dma_start(dblkF[:], sf2[1:2, :])
                        nc.gpsimd.collective_compute(
                            "AllGather", ALU.bypass,
                            replica_groups=[list(range(NCORES))],
                            ins=[dblkF[:].opt()], outs=[dgathF[:].opt()])

                    # partial h2 for this sweep's heads -> gather (sweep-0
                    # gather overlaps all of sweep 1)
                    dblk = dblk2a if sweep == 0 else dblk2b
                    fcs = (0, 1) if sweep == 0 else (2, 3)
                    for nc4 in range(4):
                        pH = PsA.tile([128, 512], F32, tag="ps_a", bufs=2,
                                      name=f"pH{sweep}_{nc4}")
                        for k, fc in enumerate(fcs):
                            nc.tensor.matmul(
                                pH[:, 0:NCLASS],
                                sxcb[:, fc, nc4 * 128:(nc4 + 1) * 128],
                                sWout[:, fc, :],
                                start=(k == 0), stop=(k == 1))
                        sh2b = P1.tile([128, NCLASS], BF16, tag="sh2b",
                                       bufs=2, name=f"sh2b{sweep}_{nc4}")
                        nc.vector.tensor_copy(sh2b[:], pH[:, 0:NCLASS])
                        nc.sync.dma_start(
                            dblk[nc4 * 128:(nc4 + 1) * 128, 0:NCLASS],
                            sh2b[:])
                    if sweep == 0:
                        nc.gpsimd.collective_compute(
                            "AllGather", ALU.bypass,
                            replica_groups=[list(range(NCORES))],
                            ins=[dblk2a[:].opt()], outs=[dgath2a[:].opt()])

            l1pool.__exit__(None, None, None)
            # ------------------------- layer 2 -------------------------
            with tc.tile_pool(name="l2", bufs=1) as P2:
                fib2 = P2.tile([128, R], BF16, name="fib2")
                sfi2b = P2.tile([1, R], BF16, name="sfi2b")
                nc.vector.tensor_copy(sfi2b[:], sfi2[:])
                nc.gpsimd.partition_broadcast(fib2[:], sfi2b[:])
                # u-tiles + sh2r skeleton built while the gather is in flight
                u2s = []
                for jb in range(8):
                    u24 = P2.tile([128, 4, 512], BF16, tag="u24", bufs=8,
                                  name=f"u24_{jb}")
                    mb = smask[:, jb * 4:(jb + 1) * 4, :]
                    _, fb = bass.broadcast_tensor_aps(
                        mb, fib2[:].rearrange("p i -> p () i"))
                    nc.vector.tensor_add(u24[:], mb, fb)
                    u2s.append(u24)

                # sh2rA/B: [0:64]=cls0-63 part, [64]=1/0, [65:129]=cls64-127
                # part, [129]=1/0.  A = heads 0-3 partial, B = heads 4-7.
                sh2rA = P2.tile([128, JC, 130], BF16, name="sh2rA")
                sh2rB = P2.tile([128, JC, 130], BF16, name="sh2rB")
                nc.vector.memset(sh2rA[:, :, 64], 1.0)
                nc.vector.memset(sh2rA[:, :, 129], 1.0)
                nc.vector.memset(sh2rB[:, :, 64], 0.0)
                nc.vector.memset(sh2rB[:, :, 129], 0.0)
                ga = dgath2a[:].rearrange("(jc p) c -> p jc c", p=128)
                nc.sync.dma_start(sh2rA[:, :, 0:64], ga[:, :, 0:64])
                nc.sync.dma_start(sh2rA[:, :, 65:129], ga[:, :, 64:128])

                # fj2 for all j from the tiny gather: [8, 512] -> [128, JC]
                sfj2b = P2.tile([128, JC], BF16, name="sfj2b")
                nc.sync.dma_start(
                    sfj2b[:],
                    dgathF[:].rearrange("c (j2 p) -> p (c j2)", p=128))
                sfj2T = P2.tile([128, JC], F32, name="sfj2T")
                nc.vector.tensor_copy(sfj2T[:], sfj2b[:])

                # big h2b gather; only the B-half matmuls wait on it
                nc.gpsimd.collective_compute(
                    "AllGather", ALU.bypass,
                    replica_groups=[list(range(NCORES))],
                    ins=[dblk2b[:].opt()], outs=[dgath2b[:].opt()])
                g = dgath2b[:].rearrange("(jc p) c -> p jc c", p=128)
                nc.sync.dma_start(sh2rB[:, :, 0:64], g[:, :, 0:64])
                nc.sync.dma_start(sh2rB[:, :, 65:129], g[:, :, 64:128])

                accA = Pagg.tile([NHID + 1, 512], F32, tag="agg0", bufs=1,
                                 name="accA")
                accB = Pagg.tile([NHID + 1, 512], F32, tag="agg1", bufs=1,
                                 name="accB")
                accZ = Pagg.tile([1, 512], F32, tag="agg2", bufs=1,
                                 name="accZ")
                onescol = P2.tile([128, 1], BF16, name="onescol")
                nc.vector.memset(onescol[:], 1.0)
                P4s = []
                for jb in range(8):
                    gt = PTN2[jb % len(PTN2)]
                    em4 = P2.tile([128, 4, 512], BF16, tag="em4b", bufs=2)
                    P4 = P2.tile([128, 4, 512], BF16, tag="p4b", bufs=8,
                                 name=f"P4b_{jb}")
                    u24 = u2s[jb]
                    if gt in ('a', 'b'):
                        for q in range(4):
                            jc = jb * 4 + q
                            nc.scalar.activation(
                                em4[:, q, :], u24[:, q, :], AF.Prelu,
                                bias=sfj2T[:, jc:jc + 1], alpha=alpha[:])
                    else:
                        w14 = P2.tile([128, 4, 512], BF16, tag="w14b", bufs=2)
                        for q in range(4):
                            jc = jb * 4 + q
                            nc.vector.tensor_scalar_add(
                                w14[:, q, :], u24[:, q, :],
                                sfj2T[:, jc:jc + 1])
                        w24 = P2.tile([128, 4, 512], BF16, tag="w24b", bufs=2)
                        nc.vector.tensor_scalar_mul(w24[:], w14[:], ALPHA)
                        nc.vector.tensor_max(em4[:], w14[:], w24[:])
                    nc.vector.tensor_scalar(
                        P4[:].bitcast(I16), em4[:], EXP_A, EXP_B,
                        op0=ALU.mult, op1=ALU.add)
                    P4s.append(P4)
                    for q in range(4):
                        jc = jb * 4 + q
                        nc.tensor.matmul(
                            accA[:], sh2rA[:, jc, 0:65], P4[:, q, :],
                            start=(jc == 0), stop=False)
                        nc.tensor.matmul(
                            accB[:], sh2rA[:, jc, 65:130], P4[:, q, :],
                            start=(jc == 0), stop=False)
                        nc.tensor.matmul(
                            accZ[:], onescol[:], P4[:, q, :],
                            start=(jc == 0), stop=(jc == JC - 1))

                # Z2 reciprocal + broadcast run while the big gather lands
                sz2 = P2.tile([1, R], F32, name="sz2")
                nc.vector.tensor_copy(sz2[:], accZ[:])
                srz2 = P2.tile([1, R], F32, name="srz2")
                srz2s = P2.tile([1, R], F32, name="srz2s")
                nc.vector.reciprocal_approx_accurate(srz2[:], sz2[:], srz2s[:])
                zb2 = P2.tile([64, R], F32, name="zb2")
                nc.gpsimd.partition_broadcast(zb2[:], srz2[:], channels=64)
                ident = P2.tile([128, 128], F32, name="ident")
                nc.sync.dma_start(ident[:], id_d.ap())

                # B-half accumulation once the big gather has landed
                for jb in range(8):
                    P4 = P4s[jb]
                    for q in range(4):
                        jc = jb * 4 + q
                        nc.tensor.matmul(
                            accA[:], sh2rB[:, jc, 0:65], P4[:, q, :],
                            start=False, stop=(jc == JC - 1))
                        nc.tensor.matmul(
                            accB[:], sh2rB[:, jc, 65:130], P4[:, q, :],
                            start=False, stop=(jc == JC - 1))

                # ---------- elu, transpose, log_softmax ----------
                # elu(x) = max(x, exp(min(x, 0)) - 1)
                halves = []
                for nmh, pg in (("a", accA[0:64, :]), ("c", accB[0:64, :])):
                    sv = P2.tile([64, R], F32, tag="sv", bufs=1,
                                 name=f"sv{nmh}")
                    nc.vector.tensor_mul(sv[:], pg, zb2[:])
                    smin = P2.tile([64, R], F32, tag="smin", bufs=1,
                                   name=f"smin{nmh}")
                    nc.vector.tensor_scalar_min(smin[:], sv[:], 0.0)
                    sex = P2.tile([64, R], F32, tag="sex", bufs=1,
                                  name=f"sex{nmh}")
                    nc.scalar.activation(sex[:], smin[:], AF.Exp)
                    sres = P2.tile([64, R], F32, tag=f"sres{nmh}", bufs=1,
                                   name=f"sres{nmh}")
                    nc.vector.scalar_tensor_tensor(
                        sres[:], sex[:], -1.0, sv[:],
                        op0=ALU.add, op1=ALU.max)
                    halves.append(sres)

                sts, negmxs, ssums = [], [], []
                for it in range(4):
                    st = P2.tile([128, 128], F32, tag="st", bufs=4,
                                 name=f"st{it}")
                    for q, sres in enumerate(halves):
                        ptp = PsS.tile([128, 64], F32, tag="ps_s", bufs=2,
                                       name=f"ptp{it}_{q}")
                        nc.tensor.transpose(
                            ptp[:], sres[:, it * 128:(it + 1) * 128],
                            ident[0:64, 0:64])
                        nc.vector.tensor_copy(
                            st[:, q * 64:(q + 1) * 64], ptp[:])
                    mx = P2.tile([128, 1], F32, tag="mx", bufs=4,
                                 name=f"mx{it}")
                    nc.vector.tensor_reduce(
                        mx[:], st[:], axis=mybir.AxisListType.X, op=ALU.max)
                    negmx = P2.tile([128, 1], F32, tag="negmx", bufs=4,
                                    name=f"negmx{it}")
                    nc.vector.tensor_scalar_mul(negmx[:], mx[:], -1.0)
                    sts.append(st); negmxs.append(negmx)
                for it in range(4):
                    sexp = P2.tile([128, 128], F32, tag="sexp", bufs=2,
                                   name=f"sexp{it}")
                    ssum = P2.tile([128, 1], F32, tag="ssum", bufs=4,
                                   name=f"ssum{it}")
                    nc.scalar.activation(
                        sexp[:], sts[it][:], AF.Exp, bias=negmxs[it][:],
                        accum_out=ssum[:])
                    ssums.append(ssum)
                slns = []
                for it in range(4):
                    sln = P2.tile([128, 1], F32, tag="sln", bufs=4,
                                  name=f"sln{it}")
                    nc.scalar.activation(sln[:], ssums[it][:], AF.Ln)
                    slns.append(sln)
                for it in range(4):
                    b2 = P2.tile([128, 1], F32, tag="b2", bufs=4,
                                 name=f"b2{it}")
                    nc.vector.tensor_sub(b2[:], negmxs[it][:], slns[it][:])
                    sout = P2.tile([128, 128], F32, tag="sout", bufs=2,
                                   name=f"sout{it}")
                    nc.scalar.activation(sout[:], sts[it][:], AF.Identity,
                                         bias=b2[:])
                    nc.sync.dma_start(
                        out_d.ap()[it * 128:(it + 1) * 128, :], sout[:])

    nc.finalize()
    return nc


def _get_nc():
    if "nc" not in _CACHE:
        _CACHE["nc"] = _build_nc()
    return _CACHE["nc"]


def kernel(**inputs):
    x = np.asarray(inputs["x"], dtype=np.float32)
    adj = np.asarray(inputs["adj"])
    W = np.asarray(inputs["W"], dtype=np.float32)
    a = np.asarray(inputs["a"], dtype=np.float32)
    W_out = np.asarray(inputs["W_out"], dtype=np.float32)
    a_out = np.asarray(inputs["a_out"], dtype=np.float32)

    bf = ml_dtypes.bfloat16
    xT = np.ascontiguousarray(x.T)                       # [512, 4096]
    xT4 = np.ascontiguousarray(
        xT.reshape(FC, 128, N).transpose(1, 0, 2)).astype(bf)
    Wcat = np.ascontiguousarray(
        W.transpose(1, 0, 2).reshape(NFEAT, 512)).astype(bf)
    WcatT = np.ascontiguousarray(
        W.transpose(1, 0, 2).reshape(NFEAT, 512).T).astype(bf)
    A12 = np.zeros((512, 16), np.float32)
    for hd in range(NHEADS):
        A12[hd * NHID:(hd + 1) * NHID, hd] = a[hd, NHID:]      # a2 -> fj
        A12[hd * NHID:(hd + 1) * NHID, 8 + hd] = a[hd, :NHID]  # a1 -> fi
    A12 = A12.astype(bf)
    WoutT = np.ascontiguousarray(W_out.T).astype(bf)
    AO = np.stack([a_out[:NCLASS], a_out[NCLASS:]], axis=1)
    AO = np.ascontiguousarray(AO).astype(bf)
    ident = np.eye(128, dtype=np.float32)
    adjm1 = (adj.astype(np.float32) - 1.0) * BIG

    in_maps = []
    for c in range(NCORES):
        r0, r1 = c * R, (c + 1) * R
        in_maps.append({
            "xT4": xT4,
            "xTblk": np.ascontiguousarray(x[r0:r1].T).astype(bf),
            "Wcat": Wcat,
            "WcatT": WcatT,
            "A12": A12,
            "Wout": W_out.astype(bf),
            "WoutT": WoutT,
            "AO": AO,
            "adjm1T": np.ascontiguousarray(adjm1[r0:r1].T).astype(bf),
            "ident": ident,
        })

    nc = _get_nc()
    trace = bool(os.environ.get("KERNEL_TRACE"))
    res = bass_utils.run_bass_kernel_spmd(
        nc, in_maps, list(range(NCORES)), trace=trace)
    kernel.last_results = res
    out = np.concatenate(
        [res.results[c]["out"] for c in range(NCORES)], axis=0)
    return np.ascontiguousarray(out, dtype=np.float32)
